# revision 1
# baseline (speedup 1.0000x reference)
"""MemristorDense forward on 8 Trainium2 NeuronCores.

Math
----
Reference computes, with R = n_in+1 rows (x plus a ones bias-row), C = 2*n_out
interleaved pos/neg columns:

    y[b,j] = 0.5 * sum_r s[b,r] * [ (Wp[r,j]+m9) * exp(L[b,r]*g_p[r,j])
                                  - (Wn[r,j]+m9) * exp(L[b,r]*g_n[r,j]) ]

where L = ln(max(2|x|,1e-12)), g = log2(n_param), m9 = max_w/9, s = sign(x).
(The k_G / K_V scalings cancel except for the m9 offset.)

Writing g = gbar + d (gbar = midrange of log2(n_param)) and Taylor-expanding
exp(L*d) = sum_k (L^k/k!) d^k turns the [B,R,C] elementwise-exp contraction
into K+1 TensorEngine matmuls:

    y = sum_k  A_k.T @ W_k,    A_k[r,b] = A_0 * (L^k/k!),  A_0 = x*(2|x|)^(gbar-1)
                               W_k[r,c] = W_0 * d^k,       W_0 = +-(W + m9)

(A_0 absorbs 0.5*s*exp(L*gbar) exactly; the minus sign of the neg columns is
folded into W_0.)  The error of truncating at k=K is weighted by
exp(L*gbar), which is tiny exactly where |L*d| is large; with K=8 the result
matches a float64 oracle to ~1e-6 relative (the fp32 reference itself only
agrees to ~3e-5).

Sharding: tensor-parallel over output columns (64 pos + 64 neg per core),
A-side replicated -- no collectives, gather is a pure concat.

Device layout: R is padded to 1152 = 9*128 rows; tiles are [128, 9*128] with
tile[p, 128*ch + c] = host_row[128*ch + p, c].  Pad rows have x=0 / W=0 /
n=2^gbar so they contribute exactly zero.  The ones bias-row is r=1024
(partition 0 of chunk 8) and flows through the same Taylor loop (its
L = ln 2, so the series converges to the exact bias current).
"""

import numpy as np

import concourse.bacc as bacc
import concourse.tile as tile
import concourse.mybir as mybir
from concourse.bass_utils import run_bass_kernel_spmd

F32 = mybir.dt.float32
ALU = mybir.AluOpType
ACT = mybir.ActivationFunctionType

NCORES = 8
B = 128
N_IN = 1024
N_OUT = 512
R = N_IN + 1
NCH = 9
RP = NCH * 128          # 1152 padded rows
CS = N_OUT // NCORES    # 64 output columns per core
KTERMS = 6              # Taylor terms k = 1..KTERMS (plus k = 0)
LN2 = 0.6931471805599453
W_SPLIT = 320           # W-update columns done on DVE; rest on GPSIMD

# Stashed by kernel() for the test harness (exec_time_ns, trace paths).
LAST_RESULTS = None


def _build_program(m9: float, gbar: float):
    nc = bacc.Bacc(
        "TRN2", target_bir_lowering=False, debug=False, num_devices=NCORES
    )
    xt_d = nc.dram_tensor("xt_in", [128, RP], F32, kind="ExternalInput").ap()
    w_d = nc.dram_tensor("w_in", [128, RP], F32, kind="ExternalInput").ap()
    n_d = nc.dram_tensor("n_in", [128, RP], F32, kind="ExternalInput").ap()
    y_d = nc.dram_tensor("y_out", [B, CS], F32, kind="ExternalOutput").ap()

    with tile.TileContext(nc) as tc:
        with (
            tc.tile_pool(name="pers", bufs=1) as pool,
            tc.tile_pool(name="apool", bufs=3) as apool,
            tc.tile_pool(name="wpool", bufs=3) as wpool,
            tc.tile_pool(name="acc", bufs=1, space="PSUM") as pspool,
            tc.tile_pool(name="tps", bufs=4, space="PSUM") as tpspool,
        ):
            eps24 = pool.tile([128, 1], F32)
            nc.gpsimd.memset(eps24[:], 1e-24)
            m9b = pool.tile([128, 1], F32)
            nc.gpsimd.memset(m9b[:], float(m9))
            xT = pool.tile([128, RP], F32)
            Nt = pool.tile([128, RP], F32)
            Lr = pool.tile([128, RP], F32)
            E1 = pool.tile([128, RP], F32)
            ysb = pool.tile([128, CS], F32)
            acc = pspool.tile([128, 2 * CS], F32)

            # Chunked input DMA so the L-chain starts before the full xT lands
            NSL = 3
            SL = RP // NSL  # 384
            for s in range(NSL):
                nc.sync.dma_start(xT[:, s * SL : (s + 1) * SL],
                                  xt_d[:, s * SL : (s + 1) * SL])
            W0 = wpool.tile([128, RP], F32, tag="w")
            nc.sync.dma_start(W0[:], w_d)
            nc.sync.dma_start(Nt[:], n_d)

            # Slice-wise: Lr = ln((2x)^2+1e-24) = 2L ; E1 = (2|x|)^(gbar-1) ;
            # A0 = x*E1 ( = 0.5*sign(x)*(2|x|)^gbar, the k=0 lhsT ).
            # Ops are grouped by activation function to avoid act-table
            # thrash on ScalarE (square/ln/exp live in different sets).
            A0 = apool.tile([128, RP], F32, tag="a")
            slices = [slice(s * SL, (s + 1) * SL) for s in range(NSL)]
            for sl in slices:
                nc.scalar.activation(E1[:, sl], xT[:, sl], ACT.Square, scale=2.0)
            for sl in slices:
                nc.scalar.activation(Lr[:, sl], E1[:, sl], ACT.Ln, bias=eps24[:])
            # Nt <- ln(n * 2^-gbar) = delta*ln2  (pad rows: exactly 0);
            # the 1/ln2 is folded into the A-update scalar below.
            nc.scalar.activation(Nt[:], Nt[:], ACT.Ln, scale=float(2.0 ** (-gbar)))
            for sl in slices:
                nc.scalar.activation(
                    E1[:, sl], Lr[:, sl], ACT.Exp, scale=(gbar - 1.0) / 2.0
                )
                nc.vector.tensor_mul(A0[:, sl], xT[:, sl], E1[:, sl])

            # W0: pos cols += m9 (ACT) ; neg cols = -(w + m9) (DVE)
            W3 = W0[:].rearrange("p (ch c) -> p ch c", c=128)
            nc.scalar.activation(
                W3[:, :, 0:CS], W3[:, :, 0:CS], ACT.Identity, bias=m9b[:]
            )
            nc.vector.tensor_scalar(
                W3[:, :, CS:128], W3[:, :, CS:128], -1.0, -float(m9),
                ALU.mult, ALU.add,
            )

            A_prev, W_prev = A0, W0
            for k in range(KTERMS + 1):
                if k > 0:
                    # A_k = A_{k-1} * (Lr*0.5) * delta-units: scalar folds the
                    # 1/2 (Lr = 2L), 1/k (factorial) and 1/ln2 (Nt = delta*ln2)
                    A_new = apool.tile([128, RP], F32, tag="a")
                    nc.vector.scalar_tensor_tensor(
                        A_new[:], A_prev[:], 0.5 / (k * LN2), Lr[:],
                        ALU.mult, ALU.mult,
                    )
                    W_new = wpool.tile([128, RP], F32, tag="w")
                    nc.vector.tensor_mul(
                        W_new[:, :W_SPLIT], W_prev[:, :W_SPLIT], Nt[:, :W_SPLIT]
                    )
                    nc.gpsimd.tensor_mul(
                        W_new[:, W_SPLIT:], W_prev[:, W_SPLIT:], Nt[:, W_SPLIT:]
                    )
                    A_prev, W_prev = A_new, W_new
                for ch in range(NCH):
                    sl = slice(ch * 128, (ch + 1) * 128)
                    nc.tensor.matmul(
                        acc[:], A_prev[:, sl], W_prev[:, sl],
                        start=(k == 0 and ch == 0),
                        stop=(k == KTERMS and ch == NCH - 1),
                    )

            yneg = pool.tile([128, CS], F32)
            nc.scalar.copy(yneg[:], acc[:, CS : 2 * CS])
            nc.vector.tensor_add(ysb[:], acc[:, 0:CS], yneg[:])
            nc.sync.dma_start(y_d, ysb[:])

    nc.compile()
    return nc


def _shard_inputs(x, w_pos, w_neg, b_pos, b_neg, n_param, gbar):
    """Per-core input maps (pure slicing / layout permutations, no flops)."""
    def swizzle(host):  # [RP, 128] -> [128, RP] device layout
        return np.ascontiguousarray(
            host.reshape(NCH, 128, 128).transpose(1, 0, 2).reshape(128, RP)
        )

    # xT[p, 128*ch + b] = x[b, 128*ch + p]; chunk 8: bias row (1.0) + zero pad
    xT = np.zeros((128, RP), np.float32)
    xT[:, : 8 * 128] = (
        x.reshape(128, 8, 128).transpose(2, 1, 0).reshape(128, 8 * 128)
    )
    xT[0, 8 * 128 :] = 1.0

    in_maps = []
    for j in range(NCORES):
        cp = slice(CS * j, CS * (j + 1))
        W_host = np.zeros((RP, 128), np.float32)
        W_host[:N_IN, 0:CS] = w_pos[:, cp]
        W_host[:N_IN, CS:128] = w_neg[:, cp]
        W_host[N_IN, 0:CS] = b_pos[cp]
        W_host[N_IN, CS:128] = b_neg[cp]
        N_host = np.full((RP, 128), 2.0 ** gbar, np.float32)
        N_host[:R, 0:CS] = n_param[:, 2 * CS * j : 2 * CS * (j + 1) : 2]
        N_host[:R, CS:128] = n_param[:, 2 * CS * j + 1 : 2 * CS * (j + 1) : 2]
        in_maps.append(
            {
                "xt_in": xT,
                "w_in": swizzle(W_host),
                "n_in": swizzle(N_host),
            }
        )
    return in_maps


def kernel(x, w_pos, w_neg, b_pos, b_neg, n_param, **run_kwargs):
    global LAST_RESULTS
    x = np.ascontiguousarray(np.asarray(x, np.float32))
    w_pos = np.asarray(w_pos, np.float32)
    w_neg = np.asarray(w_neg, np.float32)
    b_pos = np.asarray(b_pos, np.float32)
    b_neg = np.asarray(b_neg, np.float32)
    n_param = np.asarray(n_param, np.float32)

    max_w = float(
        max(w_pos.max(), w_neg.max(), b_pos.max(), b_neg.max())
    )
    m9 = max_w / 9.0
    gbar = float(0.5 * (np.log2(float(n_param.min())) + np.log2(float(n_param.max()))))

    nc = _build_program(m9, gbar)
    in_maps = _shard_inputs(x, w_pos, w_neg, b_pos, b_neg, n_param, gbar)
    res = run_bass_kernel_spmd(nc, in_maps, list(range(NCORES)), **run_kwargs)
    LAST_RESULTS = res
    return np.concatenate([res.results[j]["y_out"] for j in range(NCORES)], axis=1)



# revision 8
# speedup vs baseline: 2.0469x; 2.0469x over previous
"""MemristorDense forward on 8 Trainium2 NeuronCores.

Math
----
Reference computes, with R = n_in+1 rows (x plus a ones bias-row):

    y[b,j] = 0.5 * sum_r s[b,r] * [ (Wp[r,j]+m9) * exp(L[b,r]*g_p[r,j])
                                  - (Wn[r,j]+m9) * exp(L[b,r]*g_n[r,j]) ]

where L = ln(max(2|x|,1e-12)), g = log2(n_param), m9 = max_w/9, s = sign(x).
(The k_G / K_V scalings cancel except for the m9 offset.)

Split the exponent at EXACTLY 1 (not midrange of log2 n): g = 1 + d'.  Then
0.5*s*exp(L) = x identically -- the k=0 lhsT is the raw input tile and no
exp/power preprocessing is needed on the A side.  Taylor in u = L*d':

    y = x^T (Wp - Wn)                                  (k=0; m9 cancels)
      + sum_{k=1..3} A_k^T (Wp_k - Wn_k)               (pos|neg in one tile)
    A_k = x * (L/ln2)^k / k!,   W_k = (W+m9) * dln^k,  dln = ln(n/2)

All chain updates are plain tensor_tensor multiplies (2x DVE mode; the HW
runs scalar_tensor_tensor at 1x, so per-step scalars are banned).  The
non-geometric Taylor ratios are realized with two pre-scaled multiplier
tiles computed on the scalar engine: dh = 0.5*dln (W2,W3 steps) and
Ls3 = (2/3)*Ls (A3 step), giving k=2: 1/2 and k=3: (2/3)/4 = 1/6.

L comes from ln((2x)^2 + 1e-12) = 2L with the square done on DVE (one TT)
and the 1/2 folded into the Ls/Ls3 scales -- no abs op needed.

The k=0 term uses the exact difference wp-wn (no m9; the dominant signal
survives fp16 cleanly); its rhs is zero-padded to full width so the very
first accumulating matmul clears the PSUM has_written state.  The bias row
is EXACT: its |V|/V_ref = 2, so 2^(log2 n) = n and the contribution is
0.5*((bp+m9)*nb_p - (bn+m9)*nb_n) -- a rank-1 matmul, no transcendentals.

Eight N=512 warm-up matmuls into a scratch PSUM bank run while the DVE
chain computes, lifting the PE HAM clock gate to 2.4 GHz before the real
matmul stream issues.

Numerics (vs fp32 jax reference, CPU-simulated): rel err ~2.6e-3 with x/w
DMA'd fp16, n fp32, all chain tiles fp16, K=3.  Measured on HW: 2.7e-3.

Sharding: tensor-parallel over output columns (64 pos + 64 neg per core),
A-side replicated -- no collectives, gather is a pure concat.

Device layout: tiles are [128, 8*128] with tile[p, 128*ch + c] =
host_row[128*ch + p, c]; only the 1024 x-rows (no bias row, no padding).
"""

import numpy as np

import concourse.bacc as bacc
import concourse.tile as tile
import concourse.mybir as mybir
from concourse.bass_utils import run_bass_kernel_spmd

F32 = mybir.dt.float32
F16 = mybir.dt.float16
ALU = mybir.AluOpType
ACT = mybir.ActivationFunctionType

NCORES = 8
B = 128
N_IN = 1024
N_OUT = 512
NCH = 8
FREE = NCH * 128        # 1024
CS = N_OUT // NCORES    # 64 output columns per core
LN2 = 0.6931471805599453
NWARM = 8               # N=512 PE warm-up matmuls (~3.4us cold = HAM window)

# Stashed by kernel() for the test harness (exec_time_ns, trace paths).
LAST_RESULTS = None


def _build_program(m9: float):
    nc = bacc.Bacc(
        "TRN2", target_bir_lowering=False, debug=False, num_devices=NCORES
    )
    xt_d = nc.dram_tensor("xt_in", [128, FREE], F16, kind="ExternalInput").ap()
    w_d = nc.dram_tensor("w_in", [128, FREE], F16, kind="ExternalInput").ap()
    n_d = nc.dram_tensor("n_in", [128, FREE], F32, kind="ExternalInput").ap()
    bw_d = nc.dram_tensor("bw_in", [1, 128], F32, kind="ExternalInput").ap()
    bn_d = nc.dram_tensor("bn_in", [1, 128], F32, kind="ExternalInput").ap()
    y_d = nc.dram_tensor("y_out", [B, CS], F32, kind="ExternalOutput").ap()

    with tile.TileContext(nc) as tc:
        with (
            tc.tile_pool(name="pers", bufs=1) as pool,
            tc.tile_pool(name="acc", bufs=1, space="PSUM") as pspool,
        ):
            eps = pool.tile([128, 1], F32)
            xT = pool.tile([128, FREE], F16)
            Wt = pool.tile([128, FREE], F16)
            Nt = pool.tile([128, FREE], F32)
            Xsq = pool.tile([128, FREE], F16)
            Lt = pool.tile([128, FREE], F16)   # = 2L
            Ls = pool.tile([128, FREE], F16)   # = L/ln2
            Ls3 = pool.tile([128, FREE], F16)  # = (2/3)*L/ln2
            dln = pool.tile([128, FREE], F16)  # = ln(n/2)
            dh = pool.tile([128, FREE], F16)   # = dln/2
            W0 = pool.tile([128, FREE], F16)   # = w + m9
            A1 = pool.tile([128, FREE], F16)
            A2 = pool.tile([128, FREE], F16)
            A3 = pool.tile([128, FREE], F16)
            W1 = pool.tile([128, FREE], F16)
            W2 = pool.tile([128, FREE], F16)
            W3 = pool.tile([128, FREE], F16)
            Wd = pool.tile([128, FREE], F16)
            bw = pool.tile([1, 128], F32)
            bn = pool.tile([1, 128], F32)
            bwp = pool.tile([1, 128], F32)
            bt = pool.tile([1, 128], F32)
            bv = pool.tile([1, CS], F16)
            ones = pool.tile([1, 128], F16)
            yneg = pool.tile([128, CS], F32)
            ysb = pool.tile([128, CS], F32)
            acc = pspool.tile([128, 2 * CS], F32)
            scr = pspool.tile([128, 512], F32)  # warm-up target bank

            HG = FREE // 2
            Wd3 = Wd[:].rearrange("p (ch c) -> p ch c", c=128)
            Wt3 = Wt[:].rearrange("p (ch c) -> p ch c", c=128)

            def chunk(t, ch):
                return t[:, ch * 128:(ch + 1) * 128]

            # ---- DMA, spread across engine DGE queues for overlap:
            # sync: bias + x (feeds the A-side, earliest consumer)
            nc.sync.dma_start(bw[:], bw_d)
            nc.sync.dma_start(bn[:], bn_d)
            nc.sync.dma_start(xT[:, 0:HG], xt_d[:, 0:HG])
            nc.sync.dma_start(xT[:, HG:FREE], xt_d[:, HG:FREE])
            # scalar: n (feeds dln; ACT is idle until Xsq lands anyway)
            nc.scalar.dma_start(Nt[:, 0:HG], n_d[:, 0:HG])
            nc.scalar.dma_start(Nt[:, HG:FREE], n_d[:, HG:FREE])
            # gpsimd: eps first (Ln bias), then w
            nc.gpsimd.memset(eps[:], 1e-12)
            nc.gpsimd.dma_start(Wt[:], w_d)

            # ---- PE warm-ups: xT ch0 as both operands into the scratch bank
            for i in range(NWARM):
                nc.tensor.matmul(scr[:], chunk(xT, 0), xT[:, 0:512],
                                 start=True, stop=True)

            # ---- bias path (tiny)
            nc.vector.tensor_scalar(bwp[:], bw[:], float(m9), None, ALU.add)
            nc.gpsimd.memset(Wd3[:, :, CS:128], 0.0)
            nc.gpsimd.memset(ones[:], 0.5)
            nc.gpsimd.tensor_mul(bt[:], bwp[:], bn[:])
            nc.gpsimd.tensor_sub(bv[:], bt[:, 0:CS], bt[:, CS:128])
            # k=0 rhs: exact pos-neg difference (m9 cancels)
            nc.gpsimd.tensor_sub(Wd3[:, :, 0:CS], Wt3[:, :, 0:CS],
                                 Wt3[:, :, CS:128])

            # ---- A-side: Lt = ln((2x)^2 + 1e-12) = 2L
            nc.vector.tensor_mul(Xsq[:], xT[:], xT[:])
            nc.scalar.activation(Lt[:], Xsq[:], ACT.Ln, bias=eps[:], scale=4.0)
            nc.scalar.activation(dln[:], Nt[:], ACT.Ln, scale=0.5)
            nc.scalar.mul(dh[:], dln[:], 0.5)
            nc.scalar.mul(Ls3[:], Lt[:], 1.0 / (3.0 * LN2))

            # ---- chains, all 2x-mode tensor_tensor on DVE
            nc.vector.tensor_scalar(W0[:], Wt[:], float(m9), None, ALU.add)
            nc.vector.tensor_scalar(Ls[:], Lt[:], 0.5 / LN2, None, ALU.mult)
            nc.vector.tensor_mul(A1[:], xT[:], Ls[:])
            nc.vector.tensor_mul(W1[:], W0[:], dln[:])
            nc.vector.tensor_mul(A2[:], A1[:], Ls[:])
            nc.vector.tensor_mul(W2[:], W1[:], dh[:])
            nc.vector.tensor_mul(A3[:], A2[:], Ls3[:])
            nc.vector.tensor_mul(W3[:], W2[:], dh[:])

            # ---- real matmul stream, all accumulating into one PSUM bank
            for ch in range(NCH):   # k=0 full width (zero-padded neg half)
                nc.tensor.matmul(acc[:], chunk(xT, ch), chunk(Wd, ch),
                                 start=(ch == 0), stop=False)
            for Ak, Wk in ((A1, W1), (A2, W2), (A3, W3)):
                for ch in range(NCH):
                    nc.tensor.matmul(acc[:], chunk(Ak, ch), chunk(Wk, ch),
                                     start=False, stop=False)
            # bias: rank-1, exact (0.5 folded into the ones value)
            nc.tensor.matmul(acc[:, 0:CS], ones[:], bv[:],
                             start=False, stop=True)

            # ---- y = acc_pos - acc_neg (stage -acc_neg via ACT: DVE may
            # read only one PSUM operand)
            nc.scalar.activation(yneg[:], acc[:, CS:128], ACT.Copy,
                                 scale=-1.0)
            nc.vector.tensor_add(ysb[:], acc[:, 0:CS], yneg[:])
            nc.sync.dma_start(y_d, ysb[:])

    nc.compile()
    return nc


def _swz(host):  # [1024, 128] host-rows -> [128, 1024] device tile
    return np.ascontiguousarray(
        host.reshape(NCH, 128, 128).transpose(1, 0, 2).reshape(128, FREE)
    )


def _shard_inputs(x, w_pos, w_neg, b_pos, b_neg, n_param):
    """Per-core input maps (pure slicing / layout permutations)."""
    xT = np.ascontiguousarray(
        x.reshape(128, NCH, 128).transpose(2, 1, 0).reshape(128, FREE)
    ).astype(np.float16)

    in_maps = []
    for j in range(NCORES):
        cp = slice(CS * j, CS * (j + 1))
        Wh = np.empty((N_IN, 128), np.float16)
        Wh[:, 0:CS] = w_pos[:, cp]
        Wh[:, CS:128] = w_neg[:, cp]
        Nh = np.empty((N_IN, 128), np.float32)
        Nh[:, 0:CS] = n_param[:N_IN, 2 * CS * j:2 * CS * (j + 1):2]
        Nh[:, CS:128] = n_param[:N_IN, 2 * CS * j + 1:2 * CS * (j + 1):2]
        bwv = np.empty((1, 128), np.float32)
        bwv[0, 0:CS] = b_pos[cp]
        bwv[0, CS:128] = b_neg[cp]
        bnv = np.empty((1, 128), np.float32)
        bnv[0, 0:CS] = n_param[N_IN, 2 * CS * j:2 * CS * (j + 1):2]
        bnv[0, CS:128] = n_param[N_IN, 2 * CS * j + 1:2 * CS * (j + 1):2]
        in_maps.append({
            "xt_in": xT,
            "w_in": _swz(Wh),
            "n_in": _swz(Nh),
            "bw_in": bwv,
            "bn_in": bnv,
        })
    return in_maps


def kernel(x, w_pos, w_neg, b_pos, b_neg, n_param, **run_kwargs):
    global LAST_RESULTS
    x = np.ascontiguousarray(np.asarray(x, np.float32))
    w_pos = np.asarray(w_pos, np.float32)
    w_neg = np.asarray(w_neg, np.float32)
    b_pos = np.asarray(b_pos, np.float32)
    b_neg = np.asarray(b_neg, np.float32)
    n_param = np.asarray(n_param, np.float32)

    max_w = float(max(w_pos.max(), w_neg.max(), b_pos.max(), b_neg.max()))
    m9 = max_w / 9.0

    nc = _build_program(m9)
    in_maps = _shard_inputs(x, w_pos, w_neg, b_pos, b_neg, n_param)
    res = run_bass_kernel_spmd(nc, in_maps, list(range(NCORES)), **run_kwargs)
    LAST_RESULTS = res
    return np.concatenate(
        [res.results[j]["y_out"] for j in range(NCORES)], axis=1
    )


# revision 11
# speedup vs baseline: 2.0653x; 1.0090x over previous
"""MemristorDense forward on 8 Trainium2 NeuronCores.

Math
----
Reference computes, with R = n_in+1 rows (x plus a ones bias-row):

    y[b,j] = 0.5 * sum_r s[b,r] * [ (Wp[r,j]+m9) * exp(L[b,r]*g_p[r,j])
                                  - (Wn[r,j]+m9) * exp(L[b,r]*g_n[r,j]) ]

where L = ln(max(2|x|,1e-12)), g = log2(n_param), m9 = max_w/9, s = sign(x).
(The k_G / K_V scalings cancel except for the m9 offset.)

Split the exponent at EXACTLY 1 (not midrange of log2 n): g = 1 + d'.  Then
0.5*s*exp(L) = x identically -- the k=0 lhsT is the raw input tile and no
exp/power preprocessing is needed on the A side.  Taylor in u = L*d':

    y = x^T (Wp - Wn)                                  (k=0; m9 cancels)
      + sum_{k=1..3} A_k^T (Wp_k - Wn_k)               (pos|neg in one tile)
    A_k = x * (L/ln2)^k / k!,   W_k = (W+m9) * dln^k,  dln = ln(n/2)

All chain updates are plain tensor_tensor multiplies (2x DVE mode; the HW
runs scalar_tensor_tensor at 1x, so per-step scalars are banned).  The
non-geometric Taylor ratios are realized with two pre-scaled multiplier
tiles computed on the scalar engine: dh = 0.5*dln (W2,W3 steps) and
Ls3 = (2/3)*Ls (A3 step), giving k=2: 1/2 and k=3: (2/3)/4 = 1/6.

L comes from ln((2x)^2 + 1e-12) = 2L with the square done on DVE (one TT)
and the 1/2 folded into the Ls/Ls3 scales -- no abs op needed.

The k=0 term uses the exact difference wp-wn (no m9; the dominant signal
survives fp16 cleanly); its rhs is zero-padded to full width so the very
first accumulating matmul clears the PSUM has_written state.  The bias row
is EXACT: its |V|/V_ref = 2, so 2^(log2 n) = n and the contribution is
0.5*((bp+m9)*nb_p - (bn+m9)*nb_n) -- a rank-1 matmul, no transcendentals.

Eight N=512 warm-up matmuls into a scratch PSUM bank run while the DVE
chain computes, lifting the PE HAM clock gate to 2.4 GHz before the real
matmul stream issues.

Numerics (vs fp32 jax reference, CPU-simulated): rel err ~2.6e-3 with x/w
DMA'd fp16, n fp32, all chain tiles fp16, K=3.  Measured on HW: 2.7e-3.

Sharding: tensor-parallel over output columns (64 pos + 64 neg per core),
A-side replicated -- no collectives, gather is a pure concat.

Device layout: tiles are [128, 8*128] with tile[p, 128*ch + c] =
host_row[128*ch + p, c]; only the 1024 x-rows (no bias row, no padding).
"""

import numpy as np

import concourse.bacc as bacc
import concourse.tile as tile
import concourse.mybir as mybir
from concourse.bass_utils import run_bass_kernel_spmd

F32 = mybir.dt.float32
F16 = mybir.dt.float16
ALU = mybir.AluOpType
ACT = mybir.ActivationFunctionType

NCORES = 8
B = 128
N_IN = 1024
N_OUT = 512
NCH = 8
FREE = NCH * 128        # 1024
CS = N_OUT // NCORES    # 64 output columns per core
LN2 = 0.6931471805599453
NWARM = 8               # N=512 PE warm-up matmuls (~3.4us cold = HAM window)

# Stashed by kernel() for the test harness (exec_time_ns, trace paths).
LAST_RESULTS = None


def _build_program(m9: float):
    nc = bacc.Bacc(
        "TRN2", target_bir_lowering=False, debug=False, num_devices=NCORES
    )
    xt_d = nc.dram_tensor("xt_in", [128, FREE], F16, kind="ExternalInput").ap()
    w_d = nc.dram_tensor("w_in", [128, FREE], F16, kind="ExternalInput").ap()
    n_d = nc.dram_tensor("n_in", [128, FREE], F32, kind="ExternalInput").ap()
    bw_d = nc.dram_tensor("bw_in", [1, 128], F32, kind="ExternalInput").ap()
    bn_d = nc.dram_tensor("bn_in", [1, 128], F32, kind="ExternalInput").ap()
    y_d = nc.dram_tensor("y_out", [B, CS], F32, kind="ExternalOutput").ap()

    with tile.TileContext(nc) as tc:
        with (
            tc.tile_pool(name="pers", bufs=1) as pool,
            tc.tile_pool(name="acc", bufs=1, space="PSUM") as pspool,
        ):
            eps = pool.tile([128, 1], F32)
            xT = pool.tile([128, FREE], F16)
            Wt = pool.tile([128, FREE], F16)
            Nt = pool.tile([128, FREE], F32)
            Xsq = pool.tile([128, FREE], F16)
            Lt = pool.tile([128, FREE], F16)   # = 2L
            Ls = pool.tile([128, FREE], F16)   # = L/ln2
            Ls3 = pool.tile([128, FREE], F16)  # = (2/3)*L/ln2
            dln = pool.tile([128, FREE], F16)  # = ln(n/2)
            dh = pool.tile([128, FREE], F16)   # = dln/2
            W0 = pool.tile([128, FREE], F16)   # = w + m9
            A1 = pool.tile([128, FREE], F16)
            A2 = pool.tile([128, FREE], F16)
            A3 = pool.tile([128, FREE], F16)
            W1 = pool.tile([128, FREE], F16)
            W2 = pool.tile([128, FREE], F16)
            W3 = pool.tile([128, FREE], F16)
            Wd = pool.tile([128, FREE], F16)
            bw = pool.tile([1, 128], F32)
            bn = pool.tile([1, 128], F32)
            bwp = pool.tile([1, 128], F32)
            bt = pool.tile([1, 128], F32)
            bv = pool.tile([1, CS], F16)
            ones = pool.tile([1, 128], F16)
            yneg = pool.tile([128, CS], F32)
            ysb = pool.tile([128, CS], F32)
            acc = pspool.tile([128, 2 * CS], F32)
            scr = pspool.tile([128, 512], F32)  # warm-up target bank

            HG = FREE // 2
            Wd3 = Wd[:].rearrange("p (ch c) -> p ch c", c=128)
            Wt3 = Wt[:].rearrange("p (ch c) -> p ch c", c=128)

            def chunk(t, ch):
                return t[:, ch * 128:(ch + 1) * 128]

            # ---- DMA, spread across engine DGE queues for overlap:
            # sync: bias + x (feeds the A-side, earliest consumer)
            # full-tensor DMAs only: sliced APs of a DRAM tensor generate
            # ~600B packets (21 GB/s); contiguous transfers get ~3KB packets
            nc.sync.dma_start(xT[:], xt_d)
            nc.sync.dma_start(bw[:], bw_d)
            nc.sync.dma_start(bn[:], bn_d)
            # scalar: n (feeds dln; ACT is idle until Xsq lands anyway)
            nc.scalar.dma_start(Nt[:], n_d)
            # gpsimd: eps first (Ln bias), then w
            nc.gpsimd.memset(eps[:], 1e-12)
            nc.gpsimd.dma_start(Wt[:], w_d)

            # ---- PE warm-ups: xT ch0 as both operands into the scratch bank
            for i in range(NWARM):
                nc.tensor.matmul(scr[:], chunk(xT, 0), xT[:, 0:512],
                                 start=True, stop=True)

            # ---- bias path (tiny)
            nc.vector.tensor_scalar(bwp[:], bw[:], float(m9), None, ALU.add)
            nc.gpsimd.memset(Wd3[:, :, CS:128], 0.0)
            nc.gpsimd.memset(ones[:], 0.5)
            nc.gpsimd.tensor_mul(bt[:], bwp[:], bn[:])
            nc.gpsimd.tensor_sub(bv[:], bt[:, 0:CS], bt[:, CS:128])
            # k=0 rhs: exact pos-neg difference (m9 cancels)
            nc.gpsimd.tensor_sub(Wd3[:, :, 0:CS], Wt3[:, :, 0:CS],
                                 Wt3[:, :, CS:128])

            # ---- A-side: Lt = ln((2x)^2 + 1e-12) = 2L.  ACT runs ONLY the
            # two Ln passes: any Copy-family op would trigger a second
            # 1.28us ACT_TABLE_LOAD that delays Ln on the critical path.
            nc.vector.tensor_mul(Xsq[:], xT[:], xT[:])
            nc.scalar.activation(Lt[:], Xsq[:], ACT.Ln, bias=eps[:], scale=4.0)
            nc.scalar.activation(dln[:], Nt[:], ACT.Ln, scale=0.5)

            # ---- chains: tensor_scalar at 4x, tensor_tensor at 2x on DVE
            nc.vector.tensor_scalar(W0[:], Wt[:], float(m9), None, ALU.add)
            nc.vector.tensor_scalar(Ls[:], Lt[:], 0.5 / LN2, None, ALU.mult)
            nc.vector.tensor_scalar(Ls3[:], Lt[:], 1.0 / (3.0 * LN2), None,
                                    ALU.mult)
            nc.vector.tensor_mul(A1[:], xT[:], Ls[:])
            nc.vector.tensor_scalar(dh[:], dln[:], 0.5, None, ALU.mult)
            nc.vector.tensor_mul(W1[:], W0[:], dln[:])
            nc.vector.tensor_mul(A2[:], A1[:], Ls[:])
            nc.vector.tensor_mul(W2[:], W1[:], dh[:])
            nc.vector.tensor_mul(A3[:], A2[:], Ls3[:])
            nc.vector.tensor_mul(W3[:], W2[:], dh[:])

            # ---- real matmul stream, all accumulating into one PSUM bank
            for ch in range(NCH):   # k=0 full width (zero-padded neg half)
                nc.tensor.matmul(acc[:], chunk(xT, ch), chunk(Wd, ch),
                                 start=(ch == 0), stop=False)
            for Ak, Wk in ((A1, W1), (A2, W2), (A3, W3)):
                for ch in range(NCH):
                    nc.tensor.matmul(acc[:], chunk(Ak, ch), chunk(Wk, ch),
                                     start=False, stop=False)
            # bias: rank-1, exact (0.5 folded into the ones value)
            nc.tensor.matmul(acc[:, 0:CS], ones[:], bv[:],
                             start=False, stop=True)

            # ---- y = acc_pos - acc_neg, two DVE ops (one PSUM operand
            # each; keeps the Copy table off the ACT engine entirely)
            nc.vector.tensor_copy(yneg[:], acc[:, 0:CS])
            nc.vector.tensor_sub(ysb[:], yneg[:], acc[:, CS:128])
            nc.sync.dma_start(y_d, ysb[:])

    nc.compile()
    return nc


def _swz(host):  # [1024, 128] host-rows -> [128, 1024] device tile
    return np.ascontiguousarray(
        host.reshape(NCH, 128, 128).transpose(1, 0, 2).reshape(128, FREE)
    )


def _shard_inputs(x, w_pos, w_neg, b_pos, b_neg, n_param):
    """Per-core input maps (pure slicing / layout permutations)."""
    xT = np.ascontiguousarray(
        x.reshape(128, NCH, 128).transpose(2, 1, 0).reshape(128, FREE)
    ).astype(np.float16)

    in_maps = []
    for j in range(NCORES):
        cp = slice(CS * j, CS * (j + 1))
        Wh = np.empty((N_IN, 128), np.float16)
        Wh[:, 0:CS] = w_pos[:, cp]
        Wh[:, CS:128] = w_neg[:, cp]
        Nh = np.empty((N_IN, 128), np.float32)
        Nh[:, 0:CS] = n_param[:N_IN, 2 * CS * j:2 * CS * (j + 1):2]
        Nh[:, CS:128] = n_param[:N_IN, 2 * CS * j + 1:2 * CS * (j + 1):2]
        bwv = np.empty((1, 128), np.float32)
        bwv[0, 0:CS] = b_pos[cp]
        bwv[0, CS:128] = b_neg[cp]
        bnv = np.empty((1, 128), np.float32)
        bnv[0, 0:CS] = n_param[N_IN, 2 * CS * j:2 * CS * (j + 1):2]
        bnv[0, CS:128] = n_param[N_IN, 2 * CS * j + 1:2 * CS * (j + 1):2]
        in_maps.append({
            "xt_in": xT,
            "w_in": _swz(Wh),
            "n_in": _swz(Nh),
            "bw_in": bwv,
            "bn_in": bnv,
        })
    return in_maps


def kernel(x, w_pos, w_neg, b_pos, b_neg, n_param, **run_kwargs):
    global LAST_RESULTS
    x = np.ascontiguousarray(np.asarray(x, np.float32))
    w_pos = np.asarray(w_pos, np.float32)
    w_neg = np.asarray(w_neg, np.float32)
    b_pos = np.asarray(b_pos, np.float32)
    b_neg = np.asarray(b_neg, np.float32)
    n_param = np.asarray(n_param, np.float32)

    max_w = float(max(w_pos.max(), w_neg.max(), b_pos.max(), b_neg.max()))
    m9 = max_w / 9.0

    nc = _build_program(m9)
    in_maps = _shard_inputs(x, w_pos, w_neg, b_pos, b_neg, n_param)
    res = run_bass_kernel_spmd(nc, in_maps, list(range(NCORES)), **run_kwargs)
    LAST_RESULTS = res
    return np.concatenate(
        [res.results[j]["y_out"] for j in range(NCORES)], axis=1
    )


# revision 12
# speedup vs baseline: 2.0667x; 1.0007x over previous
"""MemristorDense forward on 8 Trainium2 NeuronCores.

Math
----
Reference computes, with R = n_in+1 rows (x plus a ones bias-row):

    y[b,j] = 0.5 * sum_r s[b,r] * [ (Wp[r,j]+m9) * exp(L[b,r]*g_p[r,j])
                                  - (Wn[r,j]+m9) * exp(L[b,r]*g_n[r,j]) ]

where L = ln(max(2|x|,1e-12)), g = log2(n_param), m9 = max_w/9, s = sign(x).
(The k_G / K_V scalings cancel except for the m9 offset.)

Split the exponent at EXACTLY 1 (not midrange of log2 n): g = 1 + d'.  Then
0.5*s*exp(L) = x identically -- the k=0 lhsT is the raw input tile and no
exp/power preprocessing is needed on the A side.  Taylor in u = L*d':

    y = x^T (Wp - Wn)                                  (k=0; m9 cancels)
      + sum_{k=1..3} A_k^T (Wp_k - Wn_k)               (pos|neg in one tile)
    A_k = x * (L/ln2)^k / k!,   W_k = (W+m9) * dln^k,  dln = ln(n/2)

All chain updates are plain tensor_tensor multiplies (2x DVE mode; the HW
runs scalar_tensor_tensor at 1x, so per-step scalars are banned).  The
non-geometric Taylor ratios are realized with two pre-scaled multiplier
tiles computed on the scalar engine: dh = 0.5*dln (W2,W3 steps) and
Ls3 = (2/3)*Ls (A3 step), giving k=2: 1/2 and k=3: (2/3)/4 = 1/6.

L comes from ln((2x)^2 + 1e-12) = 2L with the square done on DVE (one TT)
and the 1/2 folded into the Ls/Ls3 scales -- no abs op needed.

The k=0 term uses the exact difference wp-wn (no m9; the dominant signal
survives fp16 cleanly); its rhs is zero-padded to full width so the very
first accumulating matmul clears the PSUM has_written state.  The bias row
is EXACT: its |V|/V_ref = 2, so 2^(log2 n) = n and the contribution is
0.5*((bp+m9)*nb_p - (bn+m9)*nb_n) -- a rank-1 matmul, no transcendentals.

Eight N=512 warm-up matmuls into a scratch PSUM bank run while the DVE
chain computes, lifting the PE HAM clock gate to 2.4 GHz before the real
matmul stream issues.

Numerics (vs fp32 jax reference, CPU-simulated): rel err ~2.6e-3 with x/w
DMA'd fp16, n fp32, all chain tiles fp16, K=3.  Measured on HW: 2.7e-3.

Sharding: tensor-parallel over output columns (64 pos + 64 neg per core),
A-side replicated -- no collectives, gather is a pure concat.

Device layout: tiles are [128, 8*128] with tile[p, 128*ch + c] =
host_row[128*ch + p, c]; only the 1024 x-rows (no bias row, no padding).
"""

import numpy as np

import concourse.bacc as bacc
import concourse.tile as tile
import concourse.mybir as mybir
from concourse.bass_utils import run_bass_kernel_spmd

F32 = mybir.dt.float32
F16 = mybir.dt.float16
ALU = mybir.AluOpType
ACT = mybir.ActivationFunctionType

NCORES = 8
B = 128
N_IN = 1024
N_OUT = 512
NCH = 8
FREE = NCH * 128        # 1024
CS = N_OUT // NCORES    # 64 output columns per core
LN2 = 0.6931471805599453
NWARM = 8               # N=512 PE warm-up matmuls (~3.4us cold = HAM window)

# Stashed by kernel() for the test harness (exec_time_ns, trace paths).
LAST_RESULTS = None


def _build_program(m9: float):
    nc = bacc.Bacc(
        "TRN2", target_bir_lowering=False, debug=False, num_devices=NCORES
    )
    xt_d = nc.dram_tensor("xt_in", [128, FREE], F16, kind="ExternalInput").ap()
    w_d = nc.dram_tensor("w_in", [128, FREE], F16, kind="ExternalInput").ap()
    n_d = nc.dram_tensor("n_in", [128, FREE], F32, kind="ExternalInput").ap()
    bw_d = nc.dram_tensor("bw_in", [1, 128], F32, kind="ExternalInput").ap()
    bn_d = nc.dram_tensor("bn_in", [1, 128], F32, kind="ExternalInput").ap()
    y_d = nc.dram_tensor("y_out", [B, CS], F32, kind="ExternalOutput").ap()

    with tile.TileContext(nc) as tc:
        with (
            tc.tile_pool(name="pers", bufs=1) as pool,
            tc.tile_pool(name="acc", bufs=1, space="PSUM") as pspool,
        ):
            eps = pool.tile([128, 1], F32)
            xT = pool.tile([128, FREE], F16)
            Wt = pool.tile([128, FREE], F16)
            Nt = pool.tile([128, FREE], F32)
            Xsq = pool.tile([128, FREE], F16)
            Lt = pool.tile([128, FREE], F16)   # = 2L
            Ls = pool.tile([128, FREE], F16)   # = L/ln2
            Ls3 = pool.tile([128, FREE], F16)  # = (2/3)*L/ln2
            dln = pool.tile([128, FREE], F16)  # = ln(n/2)
            dh = pool.tile([128, FREE], F16)   # = dln/2
            W0 = pool.tile([128, FREE], F16)   # = w + m9
            A1 = pool.tile([128, FREE], F16)
            A2 = pool.tile([128, FREE], F16)
            A3 = pool.tile([128, FREE], F16)
            W1 = pool.tile([128, FREE], F16)
            W2 = pool.tile([128, FREE], F16)
            W3 = pool.tile([128, FREE], F16)
            Wd = pool.tile([128, FREE], F16)
            bw = pool.tile([1, 128], F32)
            bn = pool.tile([1, 128], F32)
            bwp = pool.tile([1, 128], F32)
            bt = pool.tile([1, 128], F32)
            bv = pool.tile([1, CS], F16)
            ones = pool.tile([1, 128], F16)
            yneg = pool.tile([128, CS], F32)
            ysb = pool.tile([128, CS], F32)
            acc = pspool.tile([128, 2 * CS], F32)
            scr = pspool.tile([128, 512], F32)  # warm-up target bank

            HG = FREE // 2
            Wd3 = Wd[:].rearrange("p (ch c) -> p ch c", c=128)
            Wt3 = Wt[:].rearrange("p (ch c) -> p ch c", c=128)

            def chunk(t, ch):
                return t[:, ch * 128:(ch + 1) * 128]

            # ---- DMA, spread across engine DGE queues for overlap:
            # sync: bias + x (feeds the A-side, earliest consumer)
            # full-tensor DMAs only: sliced APs of a DRAM tensor generate
            # ~600B packets (21 GB/s); contiguous transfers get ~3KB packets
            nc.sync.dma_start(xT[:], xt_d)
            nc.sync.dma_start(bw[:], bw_d)
            nc.sync.dma_start(bn[:], bn_d)
            nc.sync.dma_start(Nt[:], n_d)
            # gpsimd: eps first (Ln bias), then w
            nc.gpsimd.memset(eps[:], 1e-12)
            nc.gpsimd.dma_start(Wt[:], w_d)

            # ---- PE warm-ups: xT ch0 as both operands into the scratch bank
            for i in range(NWARM):
                nc.tensor.matmul(scr[:], chunk(xT, 0), xT[:, 0:512],
                                 start=True, stop=True)

            # ---- bias path (tiny, all DVE: gpsimd TT showed a 5us stall)
            nc.vector.tensor_scalar(bwp[:], bw[:], float(m9), None, ALU.add)
            nc.vector.tensor_mul(bt[:], bwp[:], bn[:])
            nc.vector.tensor_sub(bv[:], bt[:, 0:CS], bt[:, CS:128])
            nc.gpsimd.memset(Wd3[:, :, CS:128], 0.0)
            nc.gpsimd.memset(ones[:], 0.5)
            # k=0 rhs: exact pos-neg difference (m9 cancels)
            nc.gpsimd.tensor_sub(Wd3[:, :, 0:CS], Wt3[:, :, 0:CS],
                                 Wt3[:, :, CS:128])

            # ---- A-side: Lt = ln((2x)^2 + 1e-12) = 2L.  ACT runs ONLY the
            # two Ln passes: any Copy-family op would trigger a second
            # 1.28us ACT_TABLE_LOAD that delays Ln on the critical path.
            nc.vector.tensor_mul(Xsq[:], xT[:], xT[:])
            nc.scalar.activation(Lt[:], Xsq[:], ACT.Ln, bias=eps[:], scale=4.0)
            nc.scalar.activation(dln[:], Nt[:], ACT.Ln, scale=0.5)

            # ---- chains: tensor_scalar at 4x, tensor_tensor at 2x on DVE
            nc.vector.tensor_scalar(W0[:], Wt[:], float(m9), None, ALU.add)
            nc.vector.tensor_scalar(Ls[:], Lt[:], 0.5 / LN2, None, ALU.mult)
            nc.vector.tensor_scalar(Ls3[:], Lt[:], 1.0 / (3.0 * LN2), None,
                                    ALU.mult)
            nc.vector.tensor_mul(A1[:], xT[:], Ls[:])
            nc.vector.tensor_scalar(dh[:], dln[:], 0.5, None, ALU.mult)
            nc.vector.tensor_mul(W1[:], W0[:], dln[:])
            nc.vector.tensor_mul(A2[:], A1[:], Ls[:])
            nc.vector.tensor_mul(W2[:], W1[:], dh[:])
            nc.vector.tensor_mul(A3[:], A2[:], Ls3[:])
            nc.vector.tensor_mul(W3[:], W2[:], dh[:])

            # ---- real matmul stream, all accumulating into one PSUM bank
            for ch in range(NCH):   # k=0 full width (zero-padded neg half)
                nc.tensor.matmul(acc[:], chunk(xT, ch), chunk(Wd, ch),
                                 start=(ch == 0), stop=False)
            for Ak, Wk in ((A1, W1), (A2, W2), (A3, W3)):
                for ch in range(NCH):
                    nc.tensor.matmul(acc[:], chunk(Ak, ch), chunk(Wk, ch),
                                     start=False, stop=False)
            # bias: rank-1, exact (0.5 folded into the ones value)
            nc.tensor.matmul(acc[:, 0:CS], ones[:], bv[:],
                             start=False, stop=True)

            # ---- y = acc_pos - acc_neg, two DVE ops (one PSUM operand
            # each; keeps the Copy table off the ACT engine entirely)
            nc.vector.tensor_copy(yneg[:], acc[:, 0:CS])
            nc.vector.tensor_sub(ysb[:], yneg[:], acc[:, CS:128])
            nc.sync.dma_start(y_d, ysb[:])

    nc.compile()
    return nc


def _swz(host):  # [1024, 128] host-rows -> [128, 1024] device tile
    return np.ascontiguousarray(
        host.reshape(NCH, 128, 128).transpose(1, 0, 2).reshape(128, FREE)
    )


def _shard_inputs(x, w_pos, w_neg, b_pos, b_neg, n_param):
    """Per-core input maps (pure slicing / layout permutations)."""
    xT = np.ascontiguousarray(
        x.reshape(128, NCH, 128).transpose(2, 1, 0).reshape(128, FREE)
    ).astype(np.float16)

    in_maps = []
    for j in range(NCORES):
        cp = slice(CS * j, CS * (j + 1))
        Wh = np.empty((N_IN, 128), np.float16)
        Wh[:, 0:CS] = w_pos[:, cp]
        Wh[:, CS:128] = w_neg[:, cp]
        Nh = np.empty((N_IN, 128), np.float32)
        Nh[:, 0:CS] = n_param[:N_IN, 2 * CS * j:2 * CS * (j + 1):2]
        Nh[:, CS:128] = n_param[:N_IN, 2 * CS * j + 1:2 * CS * (j + 1):2]
        bwv = np.empty((1, 128), np.float32)
        bwv[0, 0:CS] = b_pos[cp]
        bwv[0, CS:128] = b_neg[cp]
        bnv = np.empty((1, 128), np.float32)
        bnv[0, 0:CS] = n_param[N_IN, 2 * CS * j:2 * CS * (j + 1):2]
        bnv[0, CS:128] = n_param[N_IN, 2 * CS * j + 1:2 * CS * (j + 1):2]
        in_maps.append({
            "xt_in": xT,
            "w_in": _swz(Wh),
            "n_in": _swz(Nh),
            "bw_in": bwv,
            "bn_in": bnv,
        })
    return in_maps


def kernel(x, w_pos, w_neg, b_pos, b_neg, n_param, **run_kwargs):
    global LAST_RESULTS
    x = np.ascontiguousarray(np.asarray(x, np.float32))
    w_pos = np.asarray(w_pos, np.float32)
    w_neg = np.asarray(w_neg, np.float32)
    b_pos = np.asarray(b_pos, np.float32)
    b_neg = np.asarray(b_neg, np.float32)
    n_param = np.asarray(n_param, np.float32)

    max_w = float(max(w_pos.max(), w_neg.max(), b_pos.max(), b_neg.max()))
    m9 = max_w / 9.0

    nc = _build_program(m9)
    in_maps = _shard_inputs(x, w_pos, w_neg, b_pos, b_neg, n_param)
    res = run_bass_kernel_spmd(nc, in_maps, list(range(NCORES)), **run_kwargs)
    LAST_RESULTS = res
    return np.concatenate(
        [res.results[j]["y_out"] for j in range(NCORES)], axis=1
    )


# revision 13
# speedup vs baseline: 2.2080x; 1.0683x over previous
"""MemristorDense forward on 8 Trainium2 NeuronCores.

Math
----
Reference computes, with R = n_in+1 rows (x plus a ones bias-row):

    y[b,j] = 0.5 * sum_r s[b,r] * [ (Wp[r,j]+m9) * exp(L[b,r]*g_p[r,j])
                                  - (Wn[r,j]+m9) * exp(L[b,r]*g_n[r,j]) ]

where L = ln(max(2|x|,1e-12)), g = log2(n_param), m9 = max_w/9, s = sign(x).
(The k_G / K_V scalings cancel except for the m9 offset.)

Split the exponent at EXACTLY 1 (not midrange of log2 n): g = 1 + d'.  Then
0.5*s*exp(L) = x identically -- the k=0 lhsT is the raw input tile and no
exp/power preprocessing is needed on the A side.  Taylor in u = L*d':

    y = x^T (Wp - Wn)                                  (k=0; m9 cancels)
      + sum_{k=1..3} A_k^T (Wp_k - Wn_k)               (pos|neg in one tile)
    A_k = x * (L/ln2)^k / k!,   W_k = (W+m9) * dln^k,  dln = ln(n/2)

All chain updates are plain tensor_tensor multiplies (2x DVE mode; the HW
runs scalar_tensor_tensor at 1x, so per-step scalars are banned).  The
non-geometric Taylor ratios are realized with two pre-scaled multiplier
tiles computed on the scalar engine: dh = 0.5*dln (W2,W3 steps) and
Ls3 = (2/3)*Ls (A3 step), giving k=2: 1/2 and k=3: (2/3)/4 = 1/6.

L comes from ln((2x)^2 + 1e-12) = 2L with the square done on DVE (one TT)
and the 1/2 folded into the Ls/Ls3 scales -- no abs op needed.

The k=0 term uses the exact difference wp-wn (no m9; the dominant signal
survives fp16 cleanly); its rhs is zero-padded to full width so the very
first accumulating matmul clears the PSUM has_written state.  The bias row
is EXACT: its |V|/V_ref = 2, so 2^(log2 n) = n and the contribution is
0.5*((bp+m9)*nb_p - (bn+m9)*nb_n) -- a rank-1 matmul, no transcendentals.

Eight N=512 warm-up matmuls into a scratch PSUM bank run while the DVE
chain computes, lifting the PE HAM clock gate to 2.4 GHz before the real
matmul stream issues.

Numerics (vs fp32 jax reference, CPU-simulated): rel err ~2.6e-3 with x/w
DMA'd fp16, n fp32, all chain tiles fp16, K=3.  Measured on HW: 2.7e-3.

Sharding: tensor-parallel over output columns (64 pos + 64 neg per core),
A-side replicated -- no collectives, gather is a pure concat.

Device layout: tiles are [128, 8*128] with tile[p, 128*ch + c] =
host_row[128*ch + p, c]; only the 1024 x-rows (no bias row, no padding).
"""

import numpy as np

import concourse.bacc as bacc
import concourse.tile as tile
import concourse.mybir as mybir
from concourse.bass_utils import run_bass_kernel_spmd

F32 = mybir.dt.float32
F16 = mybir.dt.float16
ALU = mybir.AluOpType
ACT = mybir.ActivationFunctionType

NCORES = 8
B = 128
N_IN = 1024
N_OUT = 512
NCH = 8
FREE = NCH * 128        # 1024
CS = N_OUT // NCORES    # 64 output columns per core
LN2 = 0.6931471805599453
NWARM = 8               # N=512 PE warm-up matmuls (~3.4us cold = HAM window)

# Stashed by kernel() for the test harness (exec_time_ns, trace paths).
LAST_RESULTS = None


def _build_program(m9: float):
    nc = bacc.Bacc(
        "TRN2", target_bir_lowering=False, debug=False, num_devices=NCORES
    )
    xt_d = nc.dram_tensor("xt_in", [128, FREE], F16, kind="ExternalInput").ap()
    w_d = nc.dram_tensor("w_in", [128, FREE], F16, kind="ExternalInput").ap()
    n_d = nc.dram_tensor("n_in", [128, FREE], F32, kind="ExternalInput").ap()
    bw_d = nc.dram_tensor("bw_in", [1, 128], F32, kind="ExternalInput").ap()
    bn_d = nc.dram_tensor("bn_in", [1, 128], F32, kind="ExternalInput").ap()
    y_d = nc.dram_tensor("y_out", [B, CS], F32, kind="ExternalOutput").ap()

    with tile.TileContext(nc) as tc:
        with (
            tc.tile_pool(name="pers", bufs=1) as pool,
            tc.tile_pool(name="acc", bufs=1, space="PSUM") as pspool,
        ):
            eps = pool.tile([128, 1], F32)
            xT = pool.tile([128, FREE], F16)
            Wt = pool.tile([128, FREE], F16)
            Nt = pool.tile([128, FREE], F32)
            Xsq = pool.tile([128, FREE], F16)
            Lt = pool.tile([128, FREE], F16)   # = 2L
            Ls = pool.tile([128, FREE], F16)   # = L/ln2
            Ls3 = pool.tile([128, FREE], F16)  # = (2/3)*L/ln2
            dln = pool.tile([128, FREE], F16)  # = ln(n/2)
            dh = pool.tile([128, FREE], F16)   # = dln/2
            W0 = pool.tile([128, FREE], F16)   # = w + m9
            A1 = pool.tile([128, FREE], F16)
            A2 = pool.tile([128, FREE], F16)
            A3 = pool.tile([128, FREE], F16)
            W1 = pool.tile([128, FREE], F16)
            W2 = pool.tile([128, FREE], F16)
            W3 = pool.tile([128, FREE], F16)
            Wd = pool.tile([128, FREE], F16)
            bw = pool.tile([1, 128], F32)
            bn = pool.tile([1, 128], F32)
            bwp = pool.tile([1, 128], F32)
            bt = pool.tile([1, 128], F32)
            bv = pool.tile([1, CS], F16)
            ones = pool.tile([1, 128], F16)
            yneg = pool.tile([128, CS], F32)
            ysb = pool.tile([128, CS], F32)
            acc = pspool.tile([128, 2 * CS], F32)
            scr = pspool.tile([128, 512], F32)  # warm-up target bank

            HG = FREE // 2
            Wd3 = Wd[:].rearrange("p (ch c) -> p ch c", c=128)
            Wt3 = Wt[:].rearrange("p (ch c) -> p ch c", c=128)

            def chunk(t, ch):
                return t[:, ch * 128:(ch + 1) * 128]

            # ---- DMA, spread across engine DGE queues for overlap:
            # sync: bias + x (feeds the A-side, earliest consumer)
            # full-tensor DMAs only: sliced APs of a DRAM tensor generate
            # ~600B packets (21 GB/s); contiguous transfers get ~3KB packets
            nc.sync.dma_start(xT[:], xt_d)
            nc.sync.dma_start(bw[:], bw_d)
            nc.sync.dma_start(bn[:], bn_d)
            nc.sync.dma_start(Nt[:], n_d)
            # gpsimd: eps first (Ln bias), then w
            nc.gpsimd.memset(eps[:], 1e-12)
            nc.gpsimd.dma_start(Wt[:], w_d)

            # ---- PE warm-ups: xT ch0 as both operands into the scratch bank
            for i in range(NWARM):
                nc.tensor.matmul(scr[:], chunk(xT, 0), xT[:, 0:512],
                                 start=True, stop=True)

            # ---- bias path (tiny, all DVE: gpsimd TT showed a 5us stall)
            nc.vector.tensor_scalar(bwp[:], bw[:], float(m9), None, ALU.add)
            nc.vector.tensor_mul(bt[:], bwp[:], bn[:])
            nc.vector.tensor_sub(bv[:], bt[:, 0:CS], bt[:, CS:128])
            nc.gpsimd.memset(Wd3[:, :, CS:128], 0.0)
            nc.gpsimd.memset(ones[:], 0.5)
            # k=0 rhs: exact pos-neg difference (m9 cancels)
            nc.gpsimd.tensor_sub(Wd3[:, :, 0:CS], Wt3[:, :, 0:CS],
                                 Wt3[:, :, CS:128])

            # ---- A-side: Lt = ln((2x)^2 + 1e-12) = 2L.  ACT runs ONLY Ln
            # ops (a second function family would cost another 1.28us
            # ACT_TABLE_LOAD).  The dummy 1-column Ln pulls the table load
            # into the DMA window instead of just before the first real Ln.
            dum = pool.tile([128, 1], F32)
            nc.scalar.activation(dum[:], eps[:], ACT.Ln)
            nc.vector.tensor_mul(Xsq[:], xT[:], xT[:])
            nc.scalar.activation(Lt[:], Xsq[:], ACT.Ln, bias=eps[:], scale=4.0)
            nc.scalar.activation(dln[:], Nt[:], ACT.Ln, scale=0.5)

            # ---- chains: tensor_scalar at 4x, tensor_tensor at 2x on DVE
            nc.vector.tensor_scalar(W0[:], Wt[:], float(m9), None, ALU.add)
            nc.vector.tensor_scalar(Ls[:], Lt[:], 0.5 / LN2, None, ALU.mult)
            nc.vector.tensor_scalar(Ls3[:], Lt[:], 1.0 / (3.0 * LN2), None,
                                    ALU.mult)
            nc.vector.tensor_mul(A1[:], xT[:], Ls[:])
            nc.vector.tensor_scalar(dh[:], dln[:], 0.5, None, ALU.mult)
            nc.vector.tensor_mul(W1[:], W0[:], dln[:])
            nc.vector.tensor_mul(A2[:], A1[:], Ls[:])
            nc.vector.tensor_mul(W2[:], W1[:], dh[:])
            nc.vector.tensor_mul(A3[:], A2[:], Ls3[:])
            nc.vector.tensor_mul(W3[:], W2[:], dh[:])

            # ---- real matmul stream, all accumulating into one PSUM bank
            for ch in range(NCH):   # k=0 full width (zero-padded neg half)
                nc.tensor.matmul(acc[:], chunk(xT, ch), chunk(Wd, ch),
                                 start=(ch == 0), stop=False)
            # bias mid-stream (rank-1, exact; 0.5 folded into ones) so the
            # accumulation group ends on the k=3 chunks, not a tail matmul
            nc.tensor.matmul(acc[:, 0:CS], ones[:], bv[:],
                             start=False, stop=False)
            for Ak, Wk in ((A1, W1), (A2, W2), (A3, W3)):
                for ch in range(NCH):
                    nc.tensor.matmul(acc[:], chunk(Ak, ch), chunk(Wk, ch),
                                     start=False,
                                     stop=(Wk is W3 and ch == NCH - 1))

            # ---- y = acc_pos - acc_neg, two DVE ops (one PSUM operand
            # each; keeps the Copy table off the ACT engine entirely)
            nc.vector.tensor_copy(yneg[:], acc[:, 0:CS])
            nc.vector.tensor_sub(ysb[:], yneg[:], acc[:, CS:128])
            nc.sync.dma_start(y_d, ysb[:])

    nc.compile()
    return nc


def _swz(host):  # [1024, 128] host-rows -> [128, 1024] device tile
    return np.ascontiguousarray(
        host.reshape(NCH, 128, 128).transpose(1, 0, 2).reshape(128, FREE)
    )


def _shard_inputs(x, w_pos, w_neg, b_pos, b_neg, n_param):
    """Per-core input maps (pure slicing / layout permutations)."""
    xT = np.ascontiguousarray(
        x.reshape(128, NCH, 128).transpose(2, 1, 0).reshape(128, FREE)
    ).astype(np.float16)

    in_maps = []
    for j in range(NCORES):
        cp = slice(CS * j, CS * (j + 1))
        Wh = np.empty((N_IN, 128), np.float16)
        Wh[:, 0:CS] = w_pos[:, cp]
        Wh[:, CS:128] = w_neg[:, cp]
        Nh = np.empty((N_IN, 128), np.float32)
        Nh[:, 0:CS] = n_param[:N_IN, 2 * CS * j:2 * CS * (j + 1):2]
        Nh[:, CS:128] = n_param[:N_IN, 2 * CS * j + 1:2 * CS * (j + 1):2]
        bwv = np.empty((1, 128), np.float32)
        bwv[0, 0:CS] = b_pos[cp]
        bwv[0, CS:128] = b_neg[cp]
        bnv = np.empty((1, 128), np.float32)
        bnv[0, 0:CS] = n_param[N_IN, 2 * CS * j:2 * CS * (j + 1):2]
        bnv[0, CS:128] = n_param[N_IN, 2 * CS * j + 1:2 * CS * (j + 1):2]
        in_maps.append({
            "xt_in": xT,
            "w_in": _swz(Wh),
            "n_in": _swz(Nh),
            "bw_in": bwv,
            "bn_in": bnv,
        })
    return in_maps


def kernel(x, w_pos, w_neg, b_pos, b_neg, n_param, **run_kwargs):
    global LAST_RESULTS
    x = np.ascontiguousarray(np.asarray(x, np.float32))
    w_pos = np.asarray(w_pos, np.float32)
    w_neg = np.asarray(w_neg, np.float32)
    b_pos = np.asarray(b_pos, np.float32)
    b_neg = np.asarray(b_neg, np.float32)
    n_param = np.asarray(n_param, np.float32)

    max_w = float(max(w_pos.max(), w_neg.max(), b_pos.max(), b_neg.max()))
    m9 = max_w / 9.0

    nc = _build_program(m9)
    in_maps = _shard_inputs(x, w_pos, w_neg, b_pos, b_neg, n_param)
    res = run_bass_kernel_spmd(nc, in_maps, list(range(NCORES)), **run_kwargs)
    LAST_RESULTS = res
    return np.concatenate(
        [res.results[j]["y_out"] for j in range(NCORES)], axis=1
    )
